# revision 1
# baseline (speedup 1.0000x reference)
"""GCN layer (normalize -> BN(eval) -> Linear -> SpMM -> LeakyReLU) on 8 TRN2 cores.

Self-contained: host-side preprocessing (sharding / edge sorting / BN folding),
Bass/Tile program builder, and SPMD runner.

Strategy (per core, SPMD):
  - nodes sharded 8 x 6250 (padded to 6272 = 49*128 local rows)
  - phase 1: X_shard = rn * (H_shard @ W') + b'   (bf16, PE matmul; row-norm via
    ones-matmul + Rsqrt; BN folded into W'/b' on host)
  - AllGather X shards -> full X [50176, 64] bf16 in DRAM
  - phase 3: edges (sorted by dest into 32-row blocks, 128-edge chunks,
    statically split per block into A/B source-half chunks for the int16
    dma_gather index range) are gathered with 128B descriptors from the
    256B-strided X table (<=1024 idx per call: SWDGE ring cap) and
    segment-summed on the PE with host-built scatter matrices S [128, 32]
    (vals folded in), accumulating in per-phase PSUM tiles (start=True
    clears has_written bank-wide); epilogue adds the phases + LeakyReLU.
"""

import os
import sys

import numpy as np

for _p in ("/opt/trn_rl_repo", "/root/.axon_site/_ro/trn_rl_repo"):
    if _p not in sys.path and os.path.isdir(_p):
        sys.path.insert(0, _p)

import ml_dtypes  # noqa: E402

BF16 = ml_dtypes.bfloat16

# ---------------- problem constants (hardcoded per contract) ----------------
N = 50000
E = 800000
DIN = 128
DOUT = 64
BN_EPS = 1e-5
SLOPE = 0.01

NCORE = 8
RPC = N // NCORE          # 6250 real rows per core
LPAD = 6272               # 49*128 padded local rows
NPAD = NCORE * LPAD       # 50176 padded global rows
BLK = 32                  # dest-window rows per block (S matrix width)
NBLK = LPAD // BLK        # 196 blocks/core (real rows end inside block 195)
BPG = 32                  # blocks per psum group (4 on partitions x 8 on free)
NGRP = (NBLK + BPG - 1) // BPG   # 7 groups (last partial: 4 blocks)
NCH_CALL = 8              # gather chunks per dma_gather call (<=1024 idx: the
                          # SWDGE descriptor ring caps one call at ~1024 descs)
OPAD = NGRP * BPG * BLK   # 7168 output rows (groups fully padded)
HALF = NPAD // 2          # 25088: int16 split point for the gather table
XW = 128                  # X rows padded to 128 bf16 cols (256B gather elems)


# ---------------- host preprocessing ----------------
def host_prep(H, rows, cols, vals, gamma, beta, run_mean, run_var, W, b):
    """Build the 8 per-core input maps + static meta (NCPB, KMAX)."""
    H = np.asarray(H, np.float32)
    rows = np.asarray(rows, np.int64)
    cols = np.asarray(cols, np.int64)
    vals = np.asarray(vals, np.float32)

    # BN fold: X = Hn @ W' + b'  with W' = diag(scale) W, b' = b + (beta-mean*scale)W
    scale = np.asarray(gamma, np.float32) / np.sqrt(np.asarray(run_var, np.float32) + BN_EPS)
    Wp = (np.asarray(W, np.float32) * scale[:, None]).astype(BF16)          # [128, 64]
    bp = (np.asarray(b, np.float32)
          + (np.asarray(beta, np.float32) - np.asarray(run_mean, np.float32) * scale)
          @ np.asarray(W, np.float32)).astype(np.float32)                    # [64]
    bp_tile = np.tile(bp[None, :], (128, 1)).astype(np.float32)              # [128, 64]

    core = rows // RPC
    lr = rows - core * RPC              # local dest row, 0..6249
    blk = lr // BLK                     # 0..195
    d = lr - blk * BLK                  # 0..31
    gcol = (cols // RPC) * LPAD + (cols % RPC)   # padded global source id
    half = (gcol >= HALF).astype(np.int64)       # 0 = table A, 1 = table B

    # ---- per-(core, block, half) slot assignment ----
    key = (core * NBLK + blk) * 2 + half
    order = np.argsort(key, kind="stable")
    counts = np.bincount(key, minlength=NCORE * NBLK * 2)
    starts = np.zeros(NCORE * NBLK * 2 + 1, np.int64)
    np.cumsum(counts, out=starts[1:])
    j_in_run = np.arange(E, dtype=np.int64) - starts[key[order]]

    c2 = counts.reshape(NCORE, NBLK, 2)
    cpa = int(np.ceil(c2[:, :, 0].max() / 128))   # A-chunks per block
    cpb = int(np.ceil(c2[:, :, 1].max() / 128))   # B-chunks per block
    ncpb = cpa + cpb
    # chunk-column layout, per psum group g (BPG blocks):
    #   [A-chunks of g's blocks (nb*cpa)] [B-chunks of g's blocks (nb*cpb)]
    # giving whole A-calls then B-calls inside each group.
    grp_nb = [min(BPG, NBLK - g * BPG) for g in range(NGRP)]
    grp_base = np.zeros(NGRP + 1, np.int64)
    np.cumsum([nb * ncpb for nb in grp_nb], out=grp_base[1:])
    nchunk = int(grp_base[-1])

    def chunk_col(b, h, k):
        # column index for chunk k of half h of block b
        g = b // BPG
        bb = b % BPG
        nb = grp_nb[g]
        if h == 0:
            return grp_base[g] + bb * cpa + k
        return grp_base[g] + nb * cpa + bb * cpb + k

    e_core = core[order]
    e_blk = blk[order]
    e_half = half[order]
    e_d = d[order]
    e_gcol = gcol[order]
    e_val = vals[order]
    e_k = j_in_run // 128
    lane = (j_in_run % 128).astype(np.int64)

    # vectorized chunk_col
    g_arr = e_blk // BPG
    bb_arr = e_blk % BPG
    nb_arr = np.array(grp_nb)[g_arr]
    col = np.where(
        e_half == 0,
        grp_base[g_arr] + bb_arr * cpa + e_k,
        grp_base[g_arr] + nb_arr * cpa + bb_arr * cpb + e_k)

    svals = np.zeros((NCORE, 128, nchunk * BLK), BF16)
    svals[e_core, lane, col * BLK + e_d] = e_val.astype(BF16)

    # slot-layout int16 idx (dummy slots use idx 0: valid, finite, S=0)
    idx = np.zeros((NCORE, 128, nchunk), np.int16)
    loc = np.where(e_half == 0, e_gcol, e_gcol - HALF)
    idx[e_core, lane, col] = loc.astype(np.int16)

    # ---- span/call layout: span = (group, half); calls of <= NCH_CALL chunks ----
    spans = []   # (col0, nchunks, table_half, wofs)
    calls = []   # per span: list of nch
    o = 0
    for g in range(NGRP):
        nb = grp_nb[g]
        for (cnt, h) in ((nb * cpa, 0), (nb * cpb, 1)):
            base = int(grp_base[g] + (0 if h == 0 else nb * cpa))
            spans.append((base, int(cnt), h, o))
            o += cnt * 8
    tot_w = o

    # wrapped-16 idx layout per call: j = c_local*128 + p;
    # tile[16k + j%16, j//16] = idx_j, replicated over the 8 16-partition groups
    idx_w = np.empty((NCORE, 128, tot_w), np.int16)
    for (c0, cnt, h, wo) in spans:
        done = 0
        while done < cnt:
            n = min(NCH_CALL, cnt - done)
            sl = idx[:, :, c0 + done:c0 + done + n]          # [NC, 128, n]
            flat = sl.transpose(0, 2, 1).reshape(NCORE, -1)  # j order
            w = flat.reshape(NCORE, n * 8, 16).transpose(0, 2, 1)
            idx_w[:, :, wo + done * 8:wo + (done + n) * 8] = np.tile(w, (1, 8, 1))
            done += n

    # ---- H transposed shards, bf16 ----
    in_maps = []
    for m in range(NCORE):
        ht = np.zeros((DIN, LPAD), BF16)
        ht[:, :RPC] = H[m * RPC:(m + 1) * RPC].T.astype(BF16)
        ht[0, RPC:] = BF16(1.0)  # pad rows get norm 1 -> finite rn, X never read
        in_maps.append(dict(
            ht=ht,
            wp=Wp,
            bp=bp_tile,
            idx=np.ascontiguousarray(idx_w[m]),
            svals=np.ascontiguousarray(svals[m]),
        ))
    meta = dict(cpa=cpa, cpb=cpb, nchunk=nchunk,
                spans=tuple(spans), tot_w=tot_w)
    return in_maps, meta


# ---------------- bass program ----------------
def _dma_gather_128(eng, out_ap, in_ap, idxs_ap, num_idxs, num_idxs_reg,
                    elem_size, elem_step):
    """bass's dma_gather minus the 256B elem assert (transpose-only per the
    ucode; non-transpose packets are byte-granular, only the table row STRIDE
    must be a multiple of 256B). DRAM-source, non-transpose only."""
    import concourse.mybir as mybir
    from concourse.bass import round_up_to_multiple, exact_div
    from concourse import ap_utils

    assert idxs_ap.dtype == mybir.dt.int16
    assert in_ap.dtype == out_ap.dtype
    assert ap_utils.ap_is_contiguous(out_ap.ap[1:])
    assert ap_utils.ap_is_contiguous(idxs_ap.ap[1:])
    assert in_ap.ap[-1][1] == out_ap.ap[-1][1] == elem_size
    assert out_ap.ap[0][1] * out_ap.ap[1][1] == round_up_to_multiple(num_idxs, 128)
    assert in_ap.ap[0][0] == elem_step
    stride_bytes = elem_step * mybir.dt.size(in_ap.dtype)
    stride_bytes_256 = exact_div(stride_bytes, 256)
    assert stride_bytes_256 < 256

    _in_ap = eng.lower_ap_dma(in_ap, for_custom_bir_dma=True)
    _idxs_ap = eng.lower_ap(idxs_ap)
    _out_ap = eng.lower_ap(out_ap)
    return eng.add_instruction(
        mybir.InstDMAGatherAnt(
            name=eng.bass.get_next_instruction_name(),
            ins=[*_in_ap, _idxs_ap, eng.lower_val_access(eng.to_reg(num_idxs_reg))],
            outs=[_out_ap],
            transpose=False,
            num_idxs=num_idxs,
            elem_size=elem_size,
            stride_bytes_256=stride_bytes_256,
            gen_mode=0,
            single_packet=True,
            queue_num=0,
            sbuf_tokens_per_rank=0,
            sbuf_free_dim_per_rank=0,
            sbuf_free_dim_pad_per_rank=0,
            sbuf_byte_offset=0,
        )
    )


def build_program(cpa, cpb, nchunk, spans, tot_w):
    import concourse.bacc as bacc
    import concourse.bass as bass
    import concourse.mybir as mybir
    from concourse.tile import TileContext

    fp32 = mybir.dt.float32
    bf16 = mybir.dt.bfloat16
    i16 = mybir.dt.int16

    ncpb = cpa + cpb
    grp_nb = [min(BPG, NBLK - g * BPG) for g in range(NGRP)]
    grp_base = [0]
    for nb in grp_nb:
        grp_base.append(grp_base[-1] + nb * ncpb)

    def col_to_chunk(col):
        g = 0
        while grp_base[g + 1] <= col:
            g += 1
        off = col - grp_base[g]
        nb = grp_nb[g]
        if off < nb * cpa:
            bb, k = divmod(off, cpa)
            h = 0
        else:
            bb, k = divmod(off - nb * cpa, cpb)
            h = 1
            k += cpa
        return g * BPG + bb, k, g

    nc = bacc.Bacc()

    ht_d = nc.declare_dram_parameter("ht", [DIN, LPAD], bf16, isOutput=False)
    wp_d = nc.declare_dram_parameter("wp", [DIN, DOUT], bf16, isOutput=False)
    bp_d = nc.declare_dram_parameter("bp", [128, DOUT], fp32, isOutput=False)
    idx_d = nc.declare_dram_parameter("idx", [128, tot_w], i16, isOutput=False)
    svals_d = nc.declare_dram_parameter("svals", [128, nchunk * BLK], bf16, isOutput=False)
    out_d = nc.declare_dram_parameter("out", [OPAD, DOUT], fp32, isOutput=True)

    xshard = nc.dram_tensor("xshard", [LPAD, XW], bf16)
    xfull = nc.dram_tensor("xfull", [NPAD, XW], bf16, addr_space="Shared")
    ss_dram = nc.dram_tensor("ss_dram", [LPAD], fp32)

    NCHK49 = LPAD // 128  # 49 phase-1 row chunks

    with TileContext(nc) as tc:
        with (
            tc.tile_pool(name="big", bufs=1) as big,
            tc.tile_pool(name="consts", bufs=1) as consts,
            tc.tile_pool(name="p1psum", bufs=1, space="PSUM") as p1ps,
            tc.tile_pool(name="xpsum", bufs=2, space="PSUM") as xps_pool,
            tc.tile_pool(name="gpsum", bufs=2, space="PSUM") as gps_pool,
            tc.tile_pool(name="gin", bufs=5) as gin,
            tc.tile_pool(name="epi", bufs=3) as epi,
        ):
            # ---------- phase 1: X shard ----------
            ones = consts.tile([128, 1], bf16)
            nc.vector.memset(ones[:], 1.0)
            wp_t = consts.tile([128, DOUT], bf16)
            nc.sync.dma_start(out=wp_t[:], in_=wp_d[:])
            bp_t = consts.tile([128, DOUT], fp32)
            nc.sync.dma_start(out=bp_t[:], in_=bp_d[:])

            ht_t = big.tile([128, LPAD], bf16)
            nc.sync.dma_start(out=ht_t[:], in_=ht_d[:])
            hsq_t = big.tile([128, LPAD], bf16)
            nc.vector.tensor_tensor(out=hsq_t[:], in0=ht_t[:], in1=ht_t[:],
                                    op=mybir.AluOpType.mult)

            ss_sb = consts.tile([1, LPAD], fp32)
            for c0 in range(0, LPAD, 512):
                w = min(512, LPAD - c0)
                ssp = p1ps.tile([1, 512], fp32, space="PSUM", tag="ssp")
                nc.tensor.matmul(out=ssp[:, :w], lhsT=ones[:], rhs=hsq_t[:, c0:c0 + w],
                                 start=True, stop=True)
                nc.vector.tensor_copy(out=ss_sb[:, c0:c0 + w], in_=ssp[:, :w])

            # reshape [1, 6272] -> [128, 49] across partitions (row c*128+p -> [p, c])
            # via a DRAM bounce (the AP balancer can't do it SBUF->SBUF)
            nc.sync.dma_start(out=ss_dram[:], in_=ss_sb[:])
            rn_in = consts.tile([128, NCHK49], fp32)
            nc.sync.dma_start(
                out=rn_in[:],
                in_=ss_dram[:].rearrange("(c p) -> p c", p=128),
            )
            sq_t = consts.tile([128, NCHK49], fp32)
            nc.scalar.activation(out=sq_t[:], in_=rn_in[:],
                                 func=mybir.ActivationFunctionType.Sqrt)
            rn_t = consts.tile([128, NCHK49], fp32)
            nc.vector.reciprocal(out=rn_t[:], in_=sq_t[:])

            xsb = big.tile([128, NCHK49 * XW], bf16)
            nc.vector.memset(xsb[:], 0.0)  # upper 64 cols of each row stay 0
            for c in range(NCHK49):
                xp = xps_pool.tile([128, DOUT], fp32, space="PSUM", tag="xp")
                nc.tensor.matmul(out=xp[:], lhsT=ht_t[:, c * 128:(c + 1) * 128],
                                 rhs=wp_t[:], start=True, stop=True)
                nc.vector.scalar_tensor_tensor(
                    out=xsb[:, c * XW:c * XW + DOUT],
                    in0=xp[:], scalar=rn_t[:, c:c + 1], in1=bp_t[:],
                    op0=mybir.AluOpType.mult, op1=mybir.AluOpType.add)

            nc.sync.dma_start(
                out=xshard[:].rearrange("(c p) f -> p c f", p=128),
                in_=xsb[:].rearrange("p (c f) -> p c f", f=XW),
            )

            # ---------- AllGather X ----------
            nc.gpsimd.collective_compute(
                "AllGather", mybir.AluOpType.bypass,
                ins=[xshard[:]], outs=[xfull[:]],
                replica_groups=[list(range(NCORE))],
            )

            # ---------- phase 3: gather + PE segment-sum ----------
            # A-phase and B-phase accumulate in separate PSUM tiles: start=True
            # clears has_written bits bank-wide, so interleaved per-block
            # accumulation windows within one bank are unsound on HW.
            grp_psA = [None] * NGRP
            grp_psB = [None] * NGRP

            def u_extent(g):
                nb = min(BPG, NBLK - g * BPG)      # blocks in this group
                return (nb + 3) // 4               # used u columns

            MAXSPAN = BPG * max(cpa, cpb)   # chunks in the largest span
            for (s0, scnt, h, wo) in spans:
                # batched idx + S-matrix loads for the whole (group, half) span
                ix_t = gin.tile([128, MAXSPAN * 8], i16, tag="ix")
                nc.scalar.dma_start(out=ix_t[:, :scnt * 8],
                                    in_=idx_d[:, wo:wo + scnt * 8])
                sv_t = gin.tile([128, MAXSPAN * BLK], bf16, tag="sv")
                nc.scalar.dma_start(out=sv_t[:, :scnt * BLK],
                                    in_=svals_d[:, s0 * BLK:(s0 + scnt) * BLK])
                table = xfull[:HALF, :DOUT] if h == 0 else xfull[HALF:, :DOUT]
                done = 0
                while done < scnt:
                    nch = min(NCH_CALL, scnt - done)
                    g_t = gin.tile([128, NCH_CALL * DOUT], bf16, tag="g")
                    gv = g_t[:, :nch * DOUT].rearrange("p (c f) -> p c f", f=DOUT)
                    _dma_gather_128(
                        nc.gpsimd, out_ap=gv, in_ap=table,
                        idxs_ap=ix_t[:, done * 8:(done + nch) * 8],
                        num_idxs=nch * 128, num_idxs_reg=nch * 128,
                        elem_size=DOUT, elem_step=XW)
                    for j in range(nch):
                        col = s0 + done + j
                        blk_i, k, g = col_to_chunk(col)
                        bb = blk_i - g * BPG
                        v, u = bb % 4, bb // 4
                        if h == 0:
                            if grp_psA[g] is None:
                                grp_psA[g] = gps_pool.tile(
                                    [128, 512], fp32, space="PSUM", tag="grpA",
                                    name=f"grpA{g}")
                            tgt = grp_psA[g]
                            start, stop = (k == 0), (k == cpa - 1)
                        else:
                            if grp_psB[g] is None:
                                grp_psB[g] = gps_pool.tile(
                                    [128, 512], fp32, space="PSUM", tag="grpB",
                                    name=f"grpB{g}")
                            tgt = grp_psB[g]
                            start, stop = (k == cpa), (k == ncpb - 1)
                        nc.tensor.matmul(
                            out=tgt[32 * v:32 * v + 32, 64 * u:64 * u + 64],
                            lhsT=sv_t[:, (done + j) * BLK:(done + j + 1) * BLK],
                            rhs=g_t[:, j * DOUT:(j + 1) * DOUT],
                            start=start, stop=stop,
                            tile_position=(0, 32 * v))

                        # group complete -> epilogue (last B-chunk of last block)
                        if bb == grp_nb[g] - 1 and k == ncpb - 1:
                            ue = u_extent(g)
                            fw = ue * DOUT
                            z_t = epi.tile([128, 8 * DOUT], fp32, tag="z")
                            nc.vector.tensor_copy(out=z_t[:, :fw],
                                                  in_=grp_psA[g][:, :fw])
                            nc.vector.tensor_tensor(
                                out=z_t[:, :fw], in0=z_t[:, :fw],
                                in1=grp_psB[g][:, :fw], op=mybir.AluOpType.add)
                            # leaky relu: max(SLOPE*x, x)
                            sc_t = epi.tile([128, 8 * DOUT], fp32, tag="sc")
                            nc.vector.tensor_scalar_mul(sc_t[:, :fw], z_t[:, :fw],
                                                        SLOPE)
                            o_t = epi.tile([128, 8 * DOUT], fp32, tag="o")
                            nc.vector.tensor_tensor(
                                out=o_t[:, :fw], in0=sc_t[:, :fw], in1=z_t[:, :fw],
                                op=mybir.AluOpType.max)
                            # rows: 1024*g + 128*u + 32*v + dd  <- sbuf [32v+dd, u*64+f]
                            nc.sync.dma_start(
                                out=out_d[:].rearrange(
                                    "(gg uu vv dd) f -> gg (vv dd) uu f",
                                    uu=8, vv=4, dd=32)[g, :, :ue, :],
                                in_=o_t[:, :fw].rearrange("p (uu f) -> p uu f", f=DOUT),
                            )

                    done += nch

    nc.compile()
    return nc


# ---------------- runner ----------------
_CACHE = {}


def _get_runner(meta):
    key = (meta["cpa"], meta["cpb"], meta["nchunk"], meta["spans"],
           meta["tot_w"])
    if key in _CACHE:
        return _CACHE[key]

    import jax
    import concourse.mybir as mybir
    from concourse import bass2jax
    from concourse.bass2jax import _bass_exec_p, partition_id_tensor
    from jax.experimental.shard_map import shard_map
    from jax.sharding import Mesh, NamedSharding, PartitionSpec

    nc = build_program(*key)
    bass2jax.install_neuronx_cc_hook()

    partition_name = nc.partition_id_tensor.name if nc.partition_id_tensor else None
    in_names, out_names, out_avals = [], [], []
    for alloc in nc.m.functions[0].allocations:
        if not isinstance(alloc, mybir.MemoryLocationSet):
            continue
        name = alloc.memorylocations[0].name
        if alloc.kind == "ExternalInput":
            if name != partition_name:
                in_names.append(name)
        elif alloc.kind == "ExternalOutput":
            out_names.append(name)
            out_avals.append(jax.core.ShapedArray(tuple(alloc.tensor_shape),
                                                  mybir.dt.np(alloc.dtype)))
    n_params = len(in_names)
    all_in = in_names + out_names
    if partition_name is not None:
        all_in.append(partition_name)

    def _body(*args):
        operands = list(args)
        if partition_name is not None:
            operands.append(partition_id_tensor())
        outs = _bass_exec_p.bind(
            *operands, out_avals=tuple(out_avals), in_names=tuple(all_in),
            out_names=tuple(out_names), lowering_input_output_aliases=(),
            sim_require_finite=False, sim_require_nnan=False, nc=nc)
        return tuple(outs)

    devices = jax.devices()[:NCORE]
    mesh = Mesh(np.asarray(devices), ("core",))
    nin = n_params + len(out_names)
    fn = jax.jit(
        shard_map(_body, mesh=mesh, in_specs=(PartitionSpec("core"),) * nin,
                  out_specs=(PartitionSpec("core"),) * len(out_names),
                  check_rep=False),
        keep_unused=True)
    sharding = NamedSharding(mesh, PartitionSpec("core"))

    runner = dict(nc=nc, fn=fn, in_names=in_names, out_names=out_names,
                  out_avals=out_avals, sharding=sharding, mesh=mesh)
    _CACHE[key] = runner
    return runner


def run_on_hw(in_maps, meta, device_args=None):
    """Execute on the 8 cores; returns (out_full [50000,64] f32, runner, device_args)."""
    import jax
    r = _get_runner(meta)
    if device_args is None:
        device_args = prepare_device_args(r, in_maps)
    outs = r["fn"](*device_args)
    jax.block_until_ready(outs)
    out = np.asarray(outs[r["out_names"].index("out")])  # [8*LPAD, 64]
    out = out.reshape(NCORE, OPAD, DOUT)[:, :RPC, :].reshape(N, DOUT)
    return out, r, device_args


def prepare_device_args(r, in_maps):
    import jax
    args = []
    for name in r["in_names"]:
        cat = np.concatenate([np.asarray(m[name]) for m in in_maps], axis=0)
        args.append(jax.device_put(cat, r["sharding"]))
    for aval in r["out_avals"]:
        z = np.zeros((NCORE * aval.shape[0], *aval.shape[1:]), aval.dtype)
        args.append(jax.device_put(z, r["sharding"]))
    return args


def kernel(H, rows, cols, vals, gamma, beta, run_mean, run_var, W, b):
    in_maps, meta = host_prep(H, rows, cols, vals, gamma, beta, run_mean, run_var, W, b)
    out, _, _ = run_on_hw(in_maps, meta)
    return out

